# revision 1
# baseline (speedup 1.0000x reference)
"""Trainium2 Bass kernel for nn_CachedCompressedLinear.

out[16, 11008] = x[16, 4096] @ ((w_q - 128) * scale).T + bias

Sharding: column-parallel over 8 NeuronCores. out_features padded
11008 -> 11264 = 8 * 1408; each core gets a [4096, 1408] int32 slice of
the (transposed) quantized weight, decodes it on-device (int32 -> bf16
with a -128 shift; integers <= 255 are exact in bf16), and computes its
[16, 1408] output slice.  x is replicated, pre-transposed and split into
bf16 hi/lo halves so the bf16 matmul carries fp32-level precision
(weights are exact in bf16, x_hi + x_lo represents x to ~2^-17).
The per-tensor scale and the bias are applied to the small f32 output
on-device in the epilogue.
"""

import sys

if "/opt/trn_rl_repo" not in sys.path:
    sys.path.insert(0, "/opt/trn_rl_repo")

import numpy as np
import ml_dtypes

IN_F = 4096
OUT_F = 11008
BATCH = 16
N_CORES = 8
O_PER = 1376  # out_features per core (11008 = 8 * 1376, no padding)
K_TILES = IN_F // 128  # 32
M = 48  # stationary columns: x_hi [0:16] | zeros [16:32] | x_lo [32:48]
# (PSUM partition reads must be 32-aligned, so lo lives at partition 32)
LO = 32
CHUNKS = [(0, 512), (512, 512), (1024, 352)]  # o-chunks within 1376

_BUILT = None


def _build():
    """Build the (SPMD, per-core) Bass program once."""
    import concourse.bass as bass
    import concourse.tile as tile
    from concourse import bacc, mybir

    dt = mybir.dt
    nc = bacc.Bacc("TRN2", target_bir_lowering=False, debug=False)

    wt = nc.dram_tensor("wt", [IN_F, O_PER], dt.int32, kind="ExternalInput")
    xt2 = nc.dram_tensor(
        "xt2", [128, (K_TILES + 1) * M], dt.bfloat16, kind="ExternalInput"
    )
    bias_rep = nc.dram_tensor(
        "bias_rep", [1, O_PER], dt.float32, kind="ExternalInput"
    )
    s_col = nc.dram_tensor("s_col", [BATCH, 1], dt.float32, kind="ExternalInput")
    out = nc.dram_tensor("out", [BATCH, O_PER], dt.float32, kind="ExternalOutput")

    PAIR = 2  # k-tiles per weight DMA (1.4 MB transfers)
    # group layout: pairs first, one single, then the final k-tile handled
    # chunk-wise (see below) so each output chunk closes as its slice lands
    GROUPS = [(g * PAIR, PAIR) for g in range(15)]  # k0..29; tail below
    BIASBLK = K_TILES  # extra stationary block holding the bias one-hot
    with tile.TileContext(nc) as tc:
        with (
            tc.tile_pool(name="consts", bufs=1) as consts,
            tc.tile_pool(name="w32", bufs=5) as w32p,
            tc.tile_pool(name="wbf", bufs=4) as wbfp,
            tc.tile_pool(name="psum", bufs=1, space=bass.MemorySpace.PSUM) as psump,
            tc.tile_pool(name="outp", bufs=1) as outp,
        ):
            # x (hi|lo) arrives host-prepacked in SBUF layout, plus one
            # extra block with the bias one-hot row: [128, 33*48]
            x_sb = consts.tile([128, (K_TILES + 1) * M], dt.bfloat16)
            nc.scalar.dma_start(x_sb[:], xt2[:])
            bias_sb = consts.tile([1, O_PER], dt.float32)
            nc.scalar.dma_start(bias_sb[:], bias_rep[:])
            s_sb = consts.tile([BATCH, 1], dt.float32)
            nc.scalar.dma_start(s_sb[:], s_col[:])

            # bias/s in bf16 hi/lo, fed to PSUM via two K=1 matmuls so the
            # epilogue does not need a separate bias add.
            rs = consts.tile([1, 1], dt.float32)
            nc.vector.reciprocal(rs[:], s_sb[0:1, 0:1])
            bq32 = consts.tile([1, O_PER], dt.float32)
            nc.vector.tensor_scalar_mul(bq32[:], bias_sb[0:1, :], rs[0:1, 0:1])
            bqh = consts.tile([1, O_PER], dt.bfloat16)
            nc.vector.tensor_copy(bqh[:], bq32[:])
            bql32 = consts.tile([1, O_PER], dt.float32)
            nc.vector.tensor_sub(bql32[:], bq32[:], bqh[:])
            bql = consts.tile([1, O_PER], dt.bfloat16)
            nc.vector.tensor_copy(bql[:], bql32[:])

            psums = [
                psump.tile([M, w], dt.float32, name=f"ps{i}", tag=f"ps{i}")
                for i, (_, w) in enumerate(CHUNKS)
            ]


            wt3 = wt[:].rearrange("(g p) f -> p g f", p=128)  # [128, 32, 1408]
            for k0, npk in GROUPS:
                wt_t = w32p.tile([128, PAIR, O_PER], dt.int32, tag="wt_t")
                nc.gpsimd.dma_start(
                    wt_t[:, 0:npk, :], wt3[:, k0 : k0 + npk, :]
                )
                wb_t = wbfp.tile([128, PAIR, O_PER], dt.bfloat16, tag="wb_t")
                for j in range(npk):
                    k = k0 + j
                    # decode: (codes - 128) cast to bf16 (exact for |v|<=256)
                    nc.vector.tensor_scalar_add(
                        wb_t[:, j, :], wt_t[:, j, :], -128.0
                    )
                    for i, (o, w) in enumerate(CHUNKS):
                        nc.tensor.matmul(
                            psums[i][:, :],
                            x_sb[:, k * M : (k + 1) * M],
                            wb_t[:, j, o : o + w],
                            start=(k == 0),
                            stop=False,
                        )
                    if k == 0:
                        # fold bias/s into the hi PSUM rows (K=1 matmuls)
                        for i, (o, w) in enumerate(CHUNKS):
                            for bvec in (bqh, bql):
                                nc.tensor.matmul(
                                    psums[i][:, :],
                                    x_sb[0:1, BIASBLK * M : (BIASBLK + 1) * M],
                                    bvec[0:1, o : o + w],
                                    start=False,
                                    stop=False,
                                )

            # final two k-tiles, chunk-wise and interleaved per chunk: each
            # output chunk's accumulation closes before the next chunk's data
            # arrives, so the PE queue at the last byte holds only the final
            # chunk's matmul (instead of ~5 queued cold matmuls)
            kA, kB = K_TILES - 2, K_TILES - 1
            wt_L = w32p.tile([128, PAIR, O_PER], dt.int32, tag="wt_t")
            wb_L = wbfp.tile([128, PAIR, O_PER], dt.bfloat16, tag="wb_t")
            for i, (o, w) in enumerate(CHUNKS):
                for j, kk in enumerate((kA, kB)):
                    nc.gpsimd.dma_start(
                        wt_L[:, j, o : o + w], wt3[:, kk, o : o + w]
                    )
                    nc.vector.tensor_scalar_add(
                        wb_L[:, j, o : o + w], wt_L[:, j, o : o + w], -128.0
                    )
                    nc.tensor.matmul(
                        psums[i][:, :],
                        x_sb[:, kk * M : (kk + 1) * M],
                        wb_L[:, j, o : o + w],
                        start=False,
                        stop=(kk == kB),
                    )

            for i, (o, w) in enumerate(CHUNKS):
                # hi -> ACT (Copy, scale fused); lo -> DVE (mul by s);
                # sum -> DVE; per-chunk output DMA. Bias is already in the
                # hi PSUM rows via the K=1 matmuls.
                his = outp.tile([BATCH, w], dt.float32, name=f"his{i}")
                nc.scalar.activation(
                    his[:],
                    psums[i][0:BATCH, :],
                    mybir.ActivationFunctionType.Copy,
                    scale=s_sb[:, 0:1],
                )
                los = outp.tile([BATCH, w], dt.float32, name=f"los{i}")
                nc.vector.tensor_scalar_mul(
                    los[:], psums[i][LO : LO + BATCH, :], s_sb[:, 0:1]
                )
                comb = outp.tile([BATCH, w], dt.float32, name=f"comb{i}")
                # all adds on DVE: GpSimd TT measured 2.2x slower and its
                # lateness blocked later out-DMAs through the sync FIFO
                nc.vector.tensor_add(comb[:], his[:], los[:])
                nc.sync.dma_start(out[:][:, o : o + w], comb[:])

    nc.compile()
    return nc


def _get_built():
    global _BUILT
    if _BUILT is None:
        _BUILT = _build()
    return _BUILT


def make_in_maps(x, w_q, scale, bias):
    """Host-side shard + layout prep. Returns per-core input dicts."""
    x = np.asarray(x, dtype=np.float32)
    w_q = np.asarray(w_q, dtype=np.int32)
    scale = np.asarray(scale, dtype=np.float32)
    bias = np.asarray(bias, dtype=np.float32)

    xT = np.ascontiguousarray(x.T)  # [4096, 16]
    xh = xT.astype(ml_dtypes.bfloat16)
    xl = (xT - xh.astype(np.float32)).astype(ml_dtypes.bfloat16)
    x48 = np.zeros((IN_F, M), dtype=ml_dtypes.bfloat16)  # [4096, 48]
    x48[:, :BATCH] = xh
    x48[:, LO : LO + BATCH] = xl
    # prepack to the SBUF layout [128, K_TILES*M]: partition p holds,
    # for each k-tile t, the stationary block row (t*128 + p)
    xt2 = np.zeros((128, (K_TILES + 1) * M), dtype=ml_dtypes.bfloat16)
    xt2[:, : K_TILES * M] = (
        x48.reshape(K_TILES, 128, M).transpose(1, 0, 2).reshape(128, K_TILES * M)
    )
    # bias one-hot block: partition 0, first BATCH stationary columns = 1
    xt2[0, K_TILES * M : K_TILES * M + BATCH] = 1.0

    s_col = np.full((BATCH, 1), scale.reshape(-1)[0], dtype=np.float32)

    in_maps = []
    for c in range(N_CORES):
        wt_c = np.ascontiguousarray(
            w_q[c * O_PER : (c + 1) * O_PER].T
        )  # [4096, 1376] int32
        bias_c = np.ascontiguousarray(
            bias[c * O_PER : (c + 1) * O_PER].reshape(1, O_PER)
        )
        in_maps.append(
            {"wt": wt_c, "xt2": xt2, "bias_rep": bias_c, "s_col": s_col}
        )
    return in_maps


def run(inputs, trace=False):
    """Run on the 8 NeuronCores. Returns (full_output, BassKernelResults)."""
    from concourse.bass_utils import run_bass_kernel_spmd

    in_maps = make_in_maps(**inputs)
    nc = _get_built()
    res = run_bass_kernel_spmd(nc, in_maps, list(range(N_CORES)), trace=trace)
    parts = [np.asarray(res.results[c]["out"]) for c in range(N_CORES)]
    full = np.concatenate(parts, axis=1)[:, :OUT_F].astype(np.float32)
    return full, res


def kernel(**inputs) -> np.ndarray:
    full, _ = run(inputs, trace=False)
    return full



# revision 2
# speedup vs baseline: 1.6731x; 1.6731x over previous
"""Trainium2 Bass kernel for nn_CachedCompressedLinear.

out[16, 11008] = x[16, 4096] @ ((w_q - 128) * scale).T + bias

Key insight vs the previous version: w_q is int32 but carries only 8
bits (codes 0..255).  The host packs codes to int8, cutting HBM traffic
4x (22.5 MB -> 5.6 MB per core).  On-device the int8 codes are decoded
to bf16 (exact: |code| <= 128 < 2^8) three ways, to spread the work:
  - a few k-tiles are decoded inline by SWDGE DMA-cast (int8 DRAM ->
    bf16 SBUF),
  - most are DMA'd raw and decoded by DVE tensor_copy (2 elem/cyc/lane),
  - the rest decoded by ACT activation-Copy (1 elem/cyc/lane).
scale is folded into x on the host (x*scale as bf16; rel-err ~2^-10,
tolerance is 2e-2), so the epilogue is a bare PSUM->SBUF copy.  The
bias is folded into PSUM via a K=2 one-hot matmul of a host-prepacked
bf16 hi/lo pair.  8 dummy warm-up matmuls run during the initial DMA
latency so the PE HAM clock is at 2.4 GHz when real work arrives.

Sharding: column-parallel over 8 cores, 1376 out-features each.
"""

import sys

if "/opt/trn_rl_repo" not in sys.path:
    sys.path.insert(0, "/opt/trn_rl_repo")

import numpy as np
import ml_dtypes

IN_F = 4096
OUT_F = 11008
BATCH = 16
N_CORES = 8
O_PER = 1376  # out_features per core (11008 = 8 * 1376)
K_TILES = IN_F // 128  # 32
M = 16  # stationary columns (batch)
CHUNKS = [(0, 512), (512, 512), (1024, 352)]  # o-chunks within 1376

# k-tile routing (by position in the PE accumulation, order is free):
#   tile 0        -> DMA-cast (first to arrive, opens the accumulation)
#   tiles 1..17   -> raw int8 DMA + DVE decode      (17 tiles)
#   tiles 18..29  -> raw int8 DMA + ACT decode      (12 tiles)
#   tiles 30..31  -> DMA-cast, issued early, held for the staggered close
DVE_SEGS = [(1, 2), (3, 4), (7, 4), (11, 4), (15, 3)]  # (start, ntiles)
ACT_SEGS = [(18, 4), (22, 4), (26, 4)]
N_RAW = 29
NWARM = 8  # dummy matmuls to warm the PE HAM clock (~3.4us at 1.2GHz)

_BUILT = None


def _build():
    import concourse.bass as bass
    import concourse.tile as tile
    from concourse import bacc, mybir

    dt = mybir.dt
    nc = bacc.Bacc("TRN2", target_bir_lowering=False, debug=False)

    # DRAM layout is fully host-prepacked: partition-major, contiguous.
    wr = nc.dram_tensor("wr", [128, N_RAW, O_PER], dt.int8, kind="ExternalInput")
    wc = nc.dram_tensor("wc", [128, 3, O_PER], dt.int8, kind="ExternalInput")
    xpk = nc.dram_tensor(
        "xpk", [128, (K_TILES + 1) * M], dt.bfloat16, kind="ExternalInput"
    )
    bias_hl = nc.dram_tensor("bias_hl", [2, O_PER], dt.bfloat16, kind="ExternalInput")
    out = nc.dram_tensor("out", [BATCH, O_PER], dt.float32, kind="ExternalOutput")

    BIASBLK = K_TILES  # stationary block holding the bias one-hot columns

    with tile.TileContext(nc) as tc:
        with (
            tc.tile_pool(name="consts", bufs=1) as consts,
            tc.tile_pool(name="w8d", bufs=3) as w8d,
            tc.tile_pool(name="w8a", bufs=2) as w8a,
            tc.tile_pool(name="wbfd", bufs=4) as wbfd,
            tc.tile_pool(name="wbfa", bufs=3) as wbfa,
            tc.tile_pool(name="wcast", bufs=1) as wcast,
            tc.tile_pool(name="psum", bufs=1, space=bass.MemorySpace.PSUM) as psump,
            tc.tile_pool(name="outp", bufs=1) as outp,
        ):
            # --- early DMAs -------------------------------------------------
            castA = wcast.tile([128, 1, O_PER], dt.bfloat16, name="castA")
            nc.gpsimd.dma_start(castA[:], wc[:][:, 0:1, :])  # k-tile 0, casts
            castB = wcast.tile([128, 2, O_PER], dt.bfloat16, name="castB")
            nc.gpsimd.dma_start(castB[:], wc[:][:, 1:3, :])  # k-tiles 30,31

            x_sb = consts.tile([128, (K_TILES + 1) * M], dt.bfloat16)
            nc.scalar.dma_start(x_sb[:], xpk[:])
            bias_sb = consts.tile([2, O_PER], dt.bfloat16)
            nc.scalar.dma_start(bias_sb[:], bias_hl[:])

            # --- PE warm-up on junk data ------------------------------------
            junkw = consts.tile([128, M], dt.bfloat16, name="junkw")
            nc.vector.memset(junkw[:], 0.0)
            junkm = consts.tile([128, 512], dt.bfloat16, name="junkm")
            nc.vector.memset(junkm[:], 0.0)

            psums = [
                psump.tile([M, w], dt.float32, name=f"ps{i}", tag=f"ps{i}")
                for i, (_, w) in enumerate(CHUNKS)
            ]
            ps_dummy = psump.tile([M, 512], dt.float32, name="psd", tag="psd")
            for _ in range(NWARM):
                nc.tensor.matmul(
                    ps_dummy[:, :], junkw[:], junkm[:], start=True, stop=True
                )

            def mm_tile(k, wsrc, start=False, stop_chunks=()):
                """The 3 chunk matmuls for one k-tile of decoded weights."""
                for i, (o, w) in enumerate(CHUNKS):
                    nc.tensor.matmul(
                        psums[i][:, :],
                        x_sb[:, k * M : (k + 1) * M],
                        wsrc[:, o : o + w],
                        start=start,
                        stop=(i in stop_chunks),
                    )

            # k-tile 0: opens the accumulation; bias folded right after.
            mm_tile(0, castA[:, 0, :], start=True)
            for i, (o, w) in enumerate(CHUNKS):
                nc.tensor.matmul(
                    psums[i][:, :],
                    x_sb[0:2, BIASBLK * M : BIASBLK * M + M],
                    bias_sb[0:2, o : o + w],
                    start=False,
                    stop=False,
                )

            # --- streaming: raw int8 DMA -> decode -> matmul ----------------
            def seg_stream(segs, rawpool, bfpool, engine):
                for k0, n in segs:
                    w8_t = rawpool.tile([128, 4, O_PER], dt.int8, tag="w8")
                    nc.sync.dma_start(
                        w8_t[:, 0:n, :], wr[:][:, k0 - 1 : k0 - 1 + n, :]
                    )
                    for j0 in range(0, n, 2):
                        nj = min(2, n - j0)
                        wb_t = bfpool.tile([128, 2, O_PER], dt.bfloat16, tag="wb")
                        if engine == "dve":
                            nc.vector.tensor_copy(
                                wb_t[:, 0:nj, :], w8_t[:, j0 : j0 + nj, :]
                            )
                        else:
                            nc.scalar.copy(
                                wb_t[:, 0:nj, :], w8_t[:, j0 : j0 + nj, :]
                            )
                        for j in range(nj):
                            mm_tile(k0 + j0 + j, wb_t[:, j, :])

            seg_stream(DVE_SEGS, w8d, wbfd, "dve")
            seg_stream(ACT_SEGS, w8a, wbfa, "act")

            # --- staggered close on the held cast tiles (k 30, 31) ----------
            for i, (o, w) in enumerate(CHUNKS):
                for j, kk in enumerate((30, 31)):
                    nc.tensor.matmul(
                        psums[i][:, :],
                        x_sb[:, kk * M : (kk + 1) * M],
                        castB[:, j, o : o + w],
                        start=False,
                        stop=(j == 1),
                    )
                # epilogue: PSUM -> SBUF f32 copy on DVE, then DMA out
                ob = outp.tile([BATCH, w], dt.float32, name=f"ob{i}")
                nc.vector.tensor_copy(ob[:], psums[i][0:BATCH, :])
                nc.sync.dma_start(out[:][:, o : o + w], ob[:])

    nc.compile()
    return nc


def _get_built():
    global _BUILT
    if _BUILT is None:
        _BUILT = _build()
    return _BUILT


def make_in_maps(x, w_q, scale, bias):
    """Host-side shard + layout prep. Returns per-core input dicts."""
    x = np.asarray(x, dtype=np.float32)
    w_q = np.asarray(w_q, dtype=np.int32)
    scale = np.asarray(scale, dtype=np.float32)
    bias = np.asarray(bias, dtype=np.float32)
    s = float(scale.reshape(-1)[0])

    # x*scale, transposed, bf16, packed partition-major + bias one-hot block
    xsT = np.ascontiguousarray((x * s).T)  # [4096, 16] f32
    xpk = np.zeros((128, (K_TILES + 1) * M), dtype=ml_dtypes.bfloat16)
    xpk[:, : K_TILES * M] = (
        xsT.reshape(K_TILES, 128, M)
        .transpose(1, 0, 2)
        .reshape(128, K_TILES * M)
        .astype(ml_dtypes.bfloat16)
    )
    xpk[0:2, K_TILES * M : K_TILES * M + BATCH] = 1.0  # bias one-hot rows

    # int8 codes, partition-major per core: [128, 32, 1376]
    w8 = (w_q - 128).astype(np.int8)

    bh32 = bias.astype(ml_dtypes.bfloat16).astype(np.float32)
    bl = (bias - bh32).astype(ml_dtypes.bfloat16)

    in_maps = []
    for c in range(N_CORES):
        wt = (
            w8[c * O_PER : (c + 1) * O_PER]
            .T.reshape(K_TILES, 128, O_PER)
            .transpose(1, 0, 2)
        )  # [128, 32, 1376]
        wr_c = np.ascontiguousarray(wt[:, 1:30, :])  # k-tiles 1..29
        wc_c = np.ascontiguousarray(wt[:, [0, 30, 31], :])  # k 0, 30, 31
        bias_hl_c = np.empty((2, O_PER), dtype=ml_dtypes.bfloat16)
        bias_hl_c[0] = bh32[c * O_PER : (c + 1) * O_PER].astype(ml_dtypes.bfloat16)
        bias_hl_c[1] = bl[c * O_PER : (c + 1) * O_PER]
        in_maps.append(
            {"wr": wr_c, "wc": wc_c, "xpk": xpk, "bias_hl": bias_hl_c}
        )
    return in_maps


def run(inputs, trace=False):
    """Run on the 8 NeuronCores. Returns (full_output, BassKernelResults)."""
    from concourse.bass_utils import run_bass_kernel_spmd

    in_maps = make_in_maps(**inputs)
    nc = _get_built()
    res = run_bass_kernel_spmd(nc, in_maps, list(range(N_CORES)), trace=trace)
    parts = [np.asarray(res.results[c]["out"]) for c in range(N_CORES)]
    full = np.concatenate(parts, axis=1).astype(np.float32)
    return full, res


def kernel(**inputs) -> np.ndarray:
    full, _ = run(inputs, trace=False)
    return full


# revision 5
# speedup vs baseline: 2.0408x; 1.2197x over previous
"""Trainium2 Bass kernel for nn_CachedCompressedLinear.

out[16, 11008] = x[16, 4096] @ ((w_q - 128) * scale).T + bias

w_q is int32 but carries only 8 bits (codes 0..255): the host packs
codes to int8, cutting HBM traffic 4x (22.5 MB -> 5.6 MB per core).
The HWDGE weight stream sustains ~420 GB/s when never stalled, so
every DMA and decode op gets its own SBUF slot (no WAR waits at all;
total SBUF use ~150 KB/partition).  int8 -> bf16 decode (exact:
|code| <= 128) is split DVE (~790 ns/tile) / ACT (~920 ns/tile) to
keep both just under the ~13.5 us stream time.

scale is folded into x on the host (x*scale as bf16; rel-err ~2^-10
vs the 2e-2 gate) so the epilogue is a bare PSUM->SBUF copy; bias is
folded into PSUM via a K=2 one-hot matmul of a host-prepacked bf16
hi/lo pair.  7 dummy warm-up matmuls bridge the ~7us framework
preamble so the PE HAM clock is warm (2.4 GHz) when real tiles arrive.

Sharding: column-parallel over 8 cores, 1376 out-features each.
"""

import sys

if "/opt/trn_rl_repo" not in sys.path:
    sys.path.insert(0, "/opt/trn_rl_repo")

import numpy as np
import ml_dtypes

IN_F = 4096
OUT_F = 11008
BATCH = 16
N_CORES = 8
O_PER = 1376  # out_features per core (11008 = 8 * 1376)
K_TILES = IN_F // 128  # 32
M = 16  # stationary columns (batch)
CHUNKS = [(0, 512), (512, 512), (1024, 352)]  # o-chunks within 1376

# Weight DMAs (k0, ntiles); first/last kept small for latency.
DMAS = [(0, 1), (1, 4), (5, 4), (9, 4), (13, 4), (17, 4), (21, 4), (25, 4), (29, 3)]
# Decode ops (k0, ntiles, engine); 2-tile ops, DVE/ACT interleaved.
OPS = [
    (0, 1, "dve"),
    (1, 2, "dve"), (3, 2, "act"),
    (5, 2, "dve"), (7, 2, "act"),
    (9, 2, "dve"), (11, 2, "act"),
    (13, 2, "dve"), (15, 2, "act"),
    (17, 2, "dve"), (19, 2, "act"),
    (21, 2, "dve"), (23, 2, "act"),
    (25, 2, "dve"), (27, 2, "act"),
    (29, 2, "dve"), (31, 1, "dve"),
]
NWARM = 7  # dummy matmuls bridging the preamble to keep the PE HAM warm

_BUILT = None


def _build():
    import concourse.bass as bass
    import concourse.tile as tile
    from concourse import bacc, mybir

    dt = mybir.dt
    nc = bacc.Bacc("TRN2", target_bir_lowering=False, debug=False)

    w8 = nc.dram_tensor("w8", [128, K_TILES, O_PER], dt.int8, kind="ExternalInput")
    xpk = nc.dram_tensor(
        "xpk", [128, (K_TILES + 1) * M], dt.bfloat16, kind="ExternalInput"
    )
    bias_hl = nc.dram_tensor("bias_hl", [2, O_PER], dt.bfloat16, kind="ExternalInput")
    out = nc.dram_tensor("out", [BATCH, O_PER], dt.float32, kind="ExternalOutput")

    BIASBLK = K_TILES

    with tile.TileContext(nc) as tc:
        with (
            tc.tile_pool(name="consts", bufs=1) as consts,
            tc.tile_pool(name="w8p", bufs=len(DMAS)) as w8p,
            tc.tile_pool(name="wbf", bufs=len(OPS)) as wbfp,
            tc.tile_pool(name="psum", bufs=1, space=bass.MemorySpace.PSUM) as psump,
            tc.tile_pool(name="outp", bufs=1) as outp,
        ):
            # opener weight DMA first, then x/bias, on the sync HWDGE ring
            w8_ts = {}
            k0, n = DMAS[0]
            w8_ts[k0] = w8p.tile([128, 4, O_PER], dt.int8, name=f"w8_{k0}", tag="w8")
            nc.sync.dma_start(w8_ts[k0][:, 0:n, :], w8[:][:, k0 : k0 + n, :])

            x_sb = consts.tile([128, (K_TILES + 1) * M], dt.bfloat16)
            nc.sync.dma_start(x_sb[:], xpk[:])
            bias_sb = consts.tile([2, O_PER], dt.bfloat16)
            nc.sync.dma_start(bias_sb[:], bias_hl[:])

            for k0, n in DMAS[1:]:
                w8_ts[k0] = w8p.tile([128, 4, O_PER], dt.int8, name=f"w8_{k0}", tag="w8")
                nc.sync.dma_start(w8_ts[k0][:, 0:n, :], w8[:][:, k0 : k0 + n, :])

            def dma_tile_for(k):
                """(tile, offset) of raw k-tile k inside its DMA tile."""
                for d0, dn in DMAS:
                    if d0 <= k < d0 + dn:
                        return w8_ts[d0], k - d0
                raise AssertionError(k)

            # PE warm-up on junk data
            junkw = consts.tile([128, M], dt.bfloat16, name="junkw")
            nc.vector.memset(junkw[:], 0.0)
            junkm = consts.tile([128, 512], dt.bfloat16, name="junkm")
            nc.vector.memset(junkm[:], 0.0)

            psums = [
                psump.tile([M, w], dt.float32, name=f"ps{i}", tag=f"ps{i}")
                for i, (_, w) in enumerate(CHUNKS)
            ]
            ps_dummy = psump.tile([M, 512], dt.float32, name="psd", tag="psd")
            for _ in range(NWARM):
                nc.tensor.matmul(
                    ps_dummy[:, :], junkw[:], junkm[:], start=True, stop=True
                )

            first = True
            wb_last = None
            for k0, n, eng in OPS:
                src_t, off = dma_tile_for(k0)
                wb_t = wbfp.tile([128, 2, O_PER], dt.bfloat16, tag="wb")
                dst = wb_t[:, 0:n, :]
                src = src_t[:, off : off + n, :]
                if eng == "dve":
                    nc.vector.tensor_copy(dst, src)
                else:
                    nc.scalar.copy(dst, src)
                for j in range(n):
                    k = k0 + j
                    if k == 31:
                        wb_last = wb_t
                        continue  # emitted in the staggered close below
                    for i, (o, w) in enumerate(CHUNKS):
                        nc.tensor.matmul(
                            psums[i][:, :],
                            x_sb[:, k * M : (k + 1) * M],
                            wb_t[:, j, o : o + w],
                            start=first,
                            stop=False,
                        )
                    if first:
                        first = False
                        for i, (o, w) in enumerate(CHUNKS):
                            nc.tensor.matmul(
                                psums[i][:, :],
                                x_sb[0:2, BIASBLK * M : BIASBLK * M + M],
                                bias_sb[0:2, o : o + w],
                                start=False,
                                stop=False,
                            )
                if k0 + n - 1 == 31:
                    wb_last = wb_t

            # staggered close: per chunk, final matmul (k=31) -> copy -> DMA.
            # Epilogue copies alternate ACT/DVE so they overlap.
            for i, (o, w) in enumerate(CHUNKS):
                nc.tensor.matmul(
                    psums[i][:, :],
                    x_sb[:, 31 * M : 32 * M],
                    wb_last[:, 0, o : o + w],
                    start=False,
                    stop=True,
                )
                ob = outp.tile([BATCH, w], dt.float32, name=f"ob{i}")
                if i == 1:
                    nc.vector.tensor_copy(ob[:], psums[i][0:BATCH, :])
                else:
                    nc.scalar.copy(ob[:], psums[i][0:BATCH, :])
                nc.sync.dma_start(out[:][:, o : o + w], ob[:])

    nc.compile()
    return nc


def _get_built():
    global _BUILT
    if _BUILT is None:
        _BUILT = _build()
    return _BUILT


def make_in_maps(x, w_q, scale, bias):
    """Host-side shard + layout prep. Returns per-core input dicts."""
    x = np.asarray(x, dtype=np.float32)
    w_q = np.asarray(w_q, dtype=np.int32)
    scale = np.asarray(scale, dtype=np.float32)
    bias = np.asarray(bias, dtype=np.float32)
    s = float(scale.reshape(-1)[0])

    xsT = np.ascontiguousarray((x * s).T)  # [4096, 16] f32
    xpk = np.zeros((128, (K_TILES + 1) * M), dtype=ml_dtypes.bfloat16)
    xpk[:, : K_TILES * M] = (
        xsT.reshape(K_TILES, 128, M)
        .transpose(1, 0, 2)
        .reshape(128, K_TILES * M)
        .astype(ml_dtypes.bfloat16)
    )
    xpk[0:2, K_TILES * M : K_TILES * M + BATCH] = 1.0  # bias one-hot rows

    w8 = (w_q - 128).astype(np.int8)

    bh32 = bias.astype(ml_dtypes.bfloat16).astype(np.float32)
    bl = (bias - bh32).astype(ml_dtypes.bfloat16)

    in_maps = []
    for c in range(N_CORES):
        wt = np.ascontiguousarray(
            w8[c * O_PER : (c + 1) * O_PER]
            .T.reshape(K_TILES, 128, O_PER)
            .transpose(1, 0, 2)
        )  # [128, 32, 1376]
        bias_hl_c = np.empty((2, O_PER), dtype=ml_dtypes.bfloat16)
        bias_hl_c[0] = bh32[c * O_PER : (c + 1) * O_PER].astype(ml_dtypes.bfloat16)
        bias_hl_c[1] = bl[c * O_PER : (c + 1) * O_PER]
        in_maps.append({"w8": wt, "xpk": xpk, "bias_hl": bias_hl_c})
    return in_maps


def run(inputs, trace=False):
    """Run on the 8 NeuronCores. Returns (full_output, BassKernelResults)."""
    from concourse.bass_utils import run_bass_kernel_spmd

    in_maps = make_in_maps(**inputs)
    nc = _get_built()
    res = run_bass_kernel_spmd(nc, in_maps, list(range(N_CORES)), trace=trace)
    parts = [np.asarray(res.results[c]["out"]) for c in range(N_CORES)]
    full = np.concatenate(parts, axis=1).astype(np.float32)
    return full, res


def kernel(**inputs) -> np.ndarray:
    full, _ = run(inputs, trace=False)
    return full
